# revision 1
# baseline (speedup 1.0000x reference)
"""3-layer GCN (gcn_norm message passing) on 8 Trainium2 NeuronCores.

Architecture (v4):
  - Nodes row-sharded across 8 cores (12500 real + 44 pad rows each); per
    layer each core computes h_mm = relu(h_prev) @ W for its shard, scaled by
    dis[src] (norm factorization: norm = dis[dest]*dis[src]), AllGathers the
    bf16 table, then aggregates messages for the destinations it owns.
  - Messages sorted by (dest-group of 4 blocks, source-quarter, dest-block).
    Per (block, quarter) runs are padded to 32-slot units so run boundaries
    are identical on all cores; one dma_gather per (group, quarter) on 4
    parallel SWDGE queues (int16 indices address the bf16 table through 4
    row-windows of 25088 rows).
  - Segment-sum on the TensorEngine: each 128-message chunk contributes one
    N=128 matmul per (statically known) destination block it overlaps, into
    a [128, 512] group PSUM tile; one-hots are 0/1 and built 8 chunks at a
    time with a single wide DVE tensor_tensor (iota_rep == seg broadcast).
  - Group epilogue: out = psum * dis[dest] (DVE wide) + bias (ACT Identity),
    relu (ACT wide), next-layer matmuls (PE), hm = psum2 * dis[own] (ACT).

All data-dependent structure is baked at trace time; the NEFF is compiled
per call and cached in-process.
"""

import os
import sys

sys.path.insert(0, "/opt/trn_rl_repo")

import numpy as np

from concourse import bacc, bass, mybir
from concourse import tile
from concourse import bass_utils

F32 = mybir.dt.float32
BF16 = mybir.dt.bfloat16
I16 = mybir.dt.int16

N_CORES = 8
NQ = 4       # source windows (int16 index range / table rows)
G = 4        # dest blocks per gather group
WOH = 8      # one-hot chunks per wide DVE op
RUN = 32     # run alignment granularity (slots)
PAD_SEG = 10000.0


def _schedule(caps32, ngrp, nblk):
    """Static layout shared by prep and builder.

    caps32: [nblk][NQ] per-(block, quarter) run capacity in RUN-slot units.
    Returns per-call offsets and the chunk->block matmul schedule.
    """
    call_cols = np.zeros(ngrp * NQ + 1, dtype=np.int64)   # gidx col base
    chunk_base = np.zeros(ngrp * NQ + 1, dtype=np.int64)  # chunk id base
    call_nidx = []
    run_slot = {}   # (b, q) -> slot offset of run inside its call
    mm_of_group = []
    for g in range(ngrp):
        blocks = list(range(g * G, min((g + 1) * G, nblk)))
        mms = []
        for q in range(NQ):
            off = 0
            spans = []
            for b in blocks:
                run_slot[(b, q)] = off
                spans.append((b, off, off + caps32[b][q] * RUN))
                off += caps32[b][q] * RUN
            nidx = ((off + 127) // 128) * 128
            call_nidx.append(nidx)
            call_cols[g * NQ + q + 1] = call_cols[g * NQ + q] + nidx // 16
            chunk_base[g * NQ + q + 1] = chunk_base[g * NQ + q] + nidx // 128
            for c in range(nidx // 128):
                lo, hi = c * 128, (c + 1) * 128
                for b, s0, s1 in spans:
                    if s0 < hi and s1 > lo:
                        mms.append((b - g * G, q, c))
        # j-major order for PSUM accumulation bracketing
        mms.sort(key=lambda m: (m[0], m[1], m[2]))
        mm_of_group.append(mms)
    mm_base = np.zeros(ngrp + 1, dtype=np.int64)
    np.cumsum([len(m) for m in mm_of_group], out=mm_base[1:])
    return {
        "call_cols": call_cols, "chunk_base": chunk_base,
        "call_nidx": call_nidx, "run_slot": run_slot,
        "mm_of_group": mm_of_group, "mm_base": mm_base,
        "n_mm": int(mm_base[-1]),
        "gidx_cols": int(call_cols[-1]),
        "total_chunks": int(chunk_base[-1]),
    }


# ----------------------------------------------------------------------------
# Host-side preparation
# ----------------------------------------------------------------------------

def _prep_inputs(x, edge_index, W0, b0, W1, b1, W2, b2, s_real):
    n = x.shape[0]
    assert n % N_CORES == 0 and s_real == n // N_CORES
    nblk = (s_real + 127) // 128
    s_pad = nblk * 128
    total = N_CORES * s_pad
    ngrp = (nblk + G - 1) // G
    assert total % NQ == 0
    wq = total // NQ
    assert wq <= 32767, f"window {wq} exceeds int16 range"

    d = np.asarray(edge_index[0], dtype=np.int64)
    s = np.asarray(edge_index[1], dtype=np.int64)

    deg = np.bincount(s, minlength=n).astype(np.float64) + 1.0
    dis = (1.0 / np.sqrt(deg)).astype(np.float32)

    dests = np.concatenate([d, np.arange(n, dtype=np.int64)])
    srcs = np.concatenate([s, np.arange(n, dtype=np.int64)])

    core = dests // s_real
    dloc = dests - core * s_real
    blk = dloc >> 7
    grp = blk // G
    jj = blk - grp * G
    sg = (srcs // s_real) * s_pad + (srcs % s_real)
    q = sg // wq
    widx = (sg - q * wq).astype(np.int64)

    # per-(core, block, quarter) counts -> shared run capacities (RUN units)
    key = (core * nblk + blk) * NQ + q
    counts = np.bincount(key, minlength=N_CORES * nblk * NQ).reshape(
        N_CORES, nblk, NQ
    )
    caps32 = np.maximum(
        (counts.max(axis=0) + RUN - 1) // RUN, 1
    )  # [nblk, NQ]

    lay = _schedule(caps32.tolist(), ngrp, nblk)

    # rank within (core, b, q)
    order = np.argsort(key, kind="stable")
    inv = np.empty_like(order)
    inv[order] = np.arange(order.size)
    starts = np.zeros(N_CORES * nblk * NQ + 1, dtype=np.int64)
    np.cumsum(counts.reshape(-1), out=starts[1:])
    rank = inv - starts[key]

    run_slot_arr = np.zeros((nblk, NQ), dtype=np.int64)
    for (b, qq), v in lay["run_slot"].items():
        run_slot_arr[b, qq] = v

    slot = run_slot_arr[blk, q] + rank               # slot within call
    call_id = grp * NQ + q
    gcol = lay["call_cols"][call_id] + (slot >> 4)
    grow = slot & 15
    gchunk = lay["chunk_base"][call_id] + (slot >> 7)
    part = slot & 127

    gidx16 = np.zeros((N_CORES, 16, lay["gidx_cols"]), dtype=np.int16)
    gidx16[core, grow, gcol] = widx.astype(np.int16)
    gidx = np.broadcast_to(
        gidx16[:, None, :, :], (N_CORES, 8, 16, lay["gidx_cols"])
    ).reshape(N_CORES, 128, lay["gidx_cols"]).copy()

    # mm col lookup: (gchunk, j) -> column
    mm_col = np.full((lay["total_chunks"], G), -1, dtype=np.int64)
    for g in range(ngrp):
        m0 = lay["mm_base"][g]
        cb = lay["chunk_base"]
        for k, (j, qq, c) in enumerate(lay["mm_of_group"][g]):
            mm_col[cb[g * NQ + qq] + c, j] = m0 + k

    meta = np.full((N_CORES, 128, lay["n_mm"]), PAD_SEG, dtype=np.float32)
    col = mm_col[gchunk, jj]
    assert (col >= 0).all()
    meta[core, part, col] = (dloc - blk * 128).astype(np.float32)

    # dense inputs
    x = np.asarray(x, dtype=np.float32)
    x_t = np.zeros((N_CORES, 128, s_pad), dtype=np.float32)
    dison = np.zeros((N_CORES, 128, nblk), dtype=np.float32)
    disd = np.zeros((N_CORES, 128, s_pad), dtype=np.float32)
    for r in range(N_CORES):
        x_t[r, :, :s_real] = x[r * s_real : (r + 1) * s_real].T
        dv = np.zeros(s_pad, dtype=np.float32)
        dv[:s_real] = dis[r * s_real : (r + 1) * s_real]
        dison[r] = dv.reshape(nblk, 128).T
        disd[r] = dv[None, :]

    wdata = np.zeros((128, 3 * 128 + 3), dtype=np.float32)
    wdata[:, 0:128] = np.asarray(W0, dtype=np.float32)
    wdata[:, 128:256] = np.asarray(W1, dtype=np.float32)
    wdata[:, 256:384] = np.asarray(W2, dtype=np.float32)
    wdata[:, 384] = np.asarray(b0, dtype=np.float32)
    wdata[:, 385] = np.asarray(b1, dtype=np.float32)
    wdata[:, 386] = np.asarray(b2, dtype=np.float32)
    iotar = np.tile(
        np.arange(128, dtype=np.float32), WOH
    )[None, :].repeat(128, axis=0)

    in_maps = [
        {
            "x_t": x_t[r], "meta": meta[r], "gidx": gidx[r],
            "wdata": wdata, "iotar": iotar, "dison": dison[r],
            "disd": disd[r],
        }
        for r in range(N_CORES)
    ]
    sched = {
        "nblk": nblk, "s_pad": s_pad, "s_real": s_real, "ngrp": ngrp,
        "caps32": caps32.tolist(),
    }
    return in_maps, sched


# ----------------------------------------------------------------------------
# Device kernel builder
# ----------------------------------------------------------------------------

def build_kernel(sched, n_cores=N_CORES):
    from contextlib import ExitStack

    nblk, s_pad, ngrp = sched["nblk"], sched["s_pad"], sched["ngrp"]
    caps32 = sched["caps32"]
    lay = _schedule(caps32, ngrp, nblk)
    total = n_cores * s_pad
    wq = total // NQ

    nc = bacc.Bacc(
        "TRN2", target_bir_lowering=False, debug=False, num_devices=n_cores,
        num_swdge_queues=NQ,
    )
    x_t = nc.dram_tensor("x_t", [128, s_pad], F32, kind="ExternalInput")
    meta = nc.dram_tensor("meta", [128, lay["n_mm"]], F32, kind="ExternalInput")
    gidx = nc.dram_tensor("gidx", [128, lay["gidx_cols"]], I16, kind="ExternalInput")
    wdata = nc.dram_tensor("wdata", [128, 3 * 128 + 3], F32, kind="ExternalInput")
    iotar = nc.dram_tensor("iotar", [128, WOH * 128], F32, kind="ExternalInput")
    dison = nc.dram_tensor("dison", [128, nblk], F32, kind="ExternalInput")
    disd = nc.dram_tensor("disd", [128, s_pad], F32, kind="ExternalInput")
    h_out = nc.dram_tensor("h_out", [128, 3 * s_pad], F32, kind="ExternalOutput")

    rg = [list(range(n_cores))]
    ID = mybir.ActivationFunctionType

    with tile.TileContext(nc) as tc, ExitStack() as ctx:
        const = ctx.enter_context(tc.tile_pool(name="const", bufs=1))
        dram = ctx.enter_context(tc.tile_pool(name="dram", bufs=1, space="DRAM"))
        xw = ctx.enter_context(tc.tile_pool(name="xw", bufs=4))
        hmm = ctx.enter_context(tc.tile_pool(name="hmm", bufs=6))
        gath = ctx.enter_context(tc.tile_pool(name="gath", bufs=2 * NQ))
        idxp = ctx.enter_context(tc.tile_pool(name="idxp", bufs=2 * NQ))
        metat = ctx.enter_context(tc.tile_pool(name="metat", bufs=3))
        ohp = ctx.enter_context(tc.tile_pool(name="ohp", bufs=8))
        outsb = ctx.enter_context(tc.tile_pool(name="outsb", bufs=2))
        ddp = ctx.enter_context(tc.tile_pool(name="ddp", bufs=2))
        rsb = ctx.enter_context(tc.tile_pool(name="rsb", bufs=2))
        agg_ps = ctx.enter_context(tc.tile_pool(name="agg_ps", bufs=2, space="PSUM"))
        mm_ps = ctx.enter_context(tc.tile_pool(name="mm_ps", bufs=2, space="PSUM"))
        mma_ps = ctx.enter_context(tc.tile_pool(name="mma_ps", bufs=2, space="PSUM"))

        ag_in = dram.tile([s_pad, 128], BF16)
        ag_outs = [
            dram.tile([total, 128], BF16, addr_space="Shared", name=f"ag_out_l{i}")
            for i in range(3)
        ]

        w_sb = const.tile([128, 3 * 128 + 3], F32)
        nc.sync.dma_start(out=w_sb[:], in_=wdata[:])
        w_bf = const.tile([128, 3 * 128], BF16)
        nc.vector.tensor_copy(w_bf[:], w_sb[:, 0 : 3 * 128])
        iota_sb = const.tile([128, WOH * 128], F32)
        nc.sync.dma_start(out=iota_sb[:], in_=iotar[:])
        dison_sb = const.tile([128, nblk], F32)
        nc.sync.dma_start(out=dison_sb[:], in_=dison[:])

        def bias(L):
            return w_sb[:, 384 + L : 385 + L]

        # ---- Phase A: table0 = (x @ W0) * dis -> ag_in ----
        for b in range(nblk):
            xt = xw.tile([128, 128], F32)
            nc.sync.dma_start(out=xt[:], in_=x_t[:, b * 128 : (b + 1) * 128])
            ps = mma_ps.tile([128, 128], F32, name="psA", tag="psA")
            nc.tensor.matmul(
                ps[:], lhsT=xt[:], rhs=w_sb[:, 0:128], start=True, stop=True
            )
            hm = hmm.tile([128, 128], BF16, name="hmA", tag="hm")
            nc.scalar.activation(
                hm[:], ps[:], ID.Copy, scale=dison_sb[:, b : b + 1]
            )
            nc.scalar.dma_start(out=ag_in[b * 128 : (b + 1) * 128, :], in_=hm[:])

        # ---- 3 layers ----
        for L in range(3):
            ag_out = ag_outs[L]
            nc.gpsimd.collective_compute(
                "AllGather",
                mybir.AluOpType.bypass,
                replica_groups=rg,
                ins=[ag_in[:].opt()],
                outs=[ag_out[:].opt()],
            )
            for g in range(ngrp):
                blocks = list(range(g * G, min((g + 1) * G, nblk)))
                nj = len(blocks)
                gts = []
                for q in range(NQ):
                    nidx = lay["call_nidx"][g * NQ + q]
                    c0 = lay["call_cols"][g * NQ + q]
                    it = idxp.tile([128, nidx // 16], I16, name="it", tag="it")
                    nc.sync.dma_start(
                        out=it[:], in_=gidx[:, c0 : c0 + nidx // 16]
                    )
                    gt = gath.tile([128, nidx], BF16, name="gt", tag="gt")
                    nc.gpsimd.dma_gather(
                        gt[:].rearrange("p (c f) -> p c f", f=128),
                        ag_out[q * wq : (q + 1) * wq, :],
                        it[:],
                        num_idxs=nidx,
                        num_idxs_reg=nidx,
                        elem_size=128,
                        elem_step=128,
                        single_packet=(nidx <= 1024),
                        queue_num=q,
                    )
                    gts.append(gt)

                mms = lay["mm_of_group"][g]
                m0 = int(lay["mm_base"][g])
                n_mm_g = len(mms)
                mt = metat.tile([128, n_mm_g], F32)
                nc.sync.dma_start(out=mt[:], in_=meta[:, m0 : m0 + n_mm_g])

                ohs = {}
                for w0 in range(0, n_mm_g, WOH):
                    wn = min(WOH, n_mm_g - w0)
                    oh = ohp.tile([128, wn * 128], BF16, name="oh", tag="oh")
                    nc.vector.tensor_tensor(
                        oh[:].rearrange("p (c f) -> p c f", f=128),
                        iota_sb[:, : wn * 128].rearrange("p (c f) -> p c f", f=128),
                        mt[:, w0 : w0 + wn].to_broadcast([128, wn, 128]),
                        mybir.AluOpType.is_equal,
                    )
                    ohs[w0] = oh

                ps = agg_ps.tile([128, G * 128], F32, name="aggps", tag="aggps")
                # first/last mm index per block j for start/stop flags
                firsts = {}
                lasts = {}
                for k, (j, qq, c) in enumerate(mms):
                    if j not in firsts:
                        firsts[j] = k
                    lasts[j] = k
                for k, (j, qq, c) in enumerate(mms):
                    cb0 = int(lay["chunk_base"][g * NQ + qq] - lay["chunk_base"][g * NQ])
                    w0 = (k // WOH) * WOH
                    off = k - w0
                    nc.tensor.matmul(
                        ps[:, j * 128 : (j + 1) * 128],
                        lhsT=gts[qq][:, c * 128 : (c + 1) * 128],
                        rhs=ohs[w0][:, off * 128 : (off + 1) * 128],
                        start=(k == firsts[j]),
                        stop=(k == lasts[j]),
                        skip_group_check=True,
                    )
                # ---- group epilogue ----
                w = nj * 128
                gb = g * G * 128
                dd = ddp.tile([128, G * 128], F32, name="dd", tag="dd")
                nc.scalar.dma_start(out=dd[:, :w], in_=disd[:, gb : gb + w])
                ob = outsb.tile([128, G * 128], F32, name="ob", tag="ob")
                nc.vector.tensor_tensor(
                    ob[:, :w], ps[:, :w], dd[:, :w], mybir.AluOpType.mult
                )
                nc.scalar.activation(ob[:, :w], ob[:, :w], ID.Identity, bias=bias(L))
                nc.sync.dma_start(
                    out=h_out[:, L * s_pad + gb : L * s_pad + gb + w],
                    in_=ob[:, :w],
                )
                if L < 2:
                    r = rsb.tile([128, G * 128], BF16, name="r", tag="r")
                    nc.scalar.activation(r[:, :w], ob[:, :w], ID.Relu)
                    ps2 = mm_ps.tile([128, G * 128], F32, name="ps2", tag="ps2")
                    for j in range(nj):
                        nc.tensor.matmul(
                            ps2[:, j * 128 : (j + 1) * 128],
                            lhsT=r[:, j * 128 : (j + 1) * 128],
                            rhs=w_bf[:, (L + 1) * 128 : (L + 2) * 128],
                            start=True,
                            stop=True,
                            skip_group_check=True,
                        )
                    for j in range(nj):
                        b = blocks[j]
                        hm = hmm.tile([128, 128], BF16, name="hm", tag="hm")
                        nc.scalar.activation(
                            hm[:], ps2[:, j * 128 : (j + 1) * 128], ID.Copy,
                            scale=dison_sb[:, b : b + 1],
                        )
                        nc.scalar.dma_start(
                            out=ag_in[b * 128 : (b + 1) * 128, :], in_=hm[:]
                        )

    nc.compile()
    return nc


_BUILD_CACHE = {}


def _get_kernel(sched):
    key = (
        sched["nblk"], sched["s_pad"],
        tuple(tuple(c) for c in sched["caps32"]),
    )
    if key not in _BUILD_CACHE:
        _BUILD_CACHE[key] = build_kernel(sched)
    return _BUILD_CACHE[key]


# ----------------------------------------------------------------------------
# Entry point
# ----------------------------------------------------------------------------

def _run(x, edge_index, W0, b0, W1, b1, W2, b2, trace=False):
    n = int(np.asarray(x).shape[0])
    s_real = n // N_CORES
    in_maps, sched = _prep_inputs(
        x, edge_index, W0, b0, W1, b1, W2, b2, s_real
    )
    s_pad = sched["s_pad"]
    nc = _get_kernel(sched)
    res = bass_utils.run_bass_kernel_spmd(
        nc, in_maps, core_ids=list(range(N_CORES)), trace=trace
    )
    outs = []
    for L in range(3):
        h = np.concatenate(
            [
                res.results[r]["h_out"][:, L * s_pad : L * s_pad + s_real]
                for r in range(N_CORES)
            ],
            axis=1,
        ).T
        outs.append(h)
    full = np.stack(outs, axis=1).astype(np.float32)
    return full, res


def kernel(**inputs):
    trace = os.environ.get("TRN_KERNEL_TRACE", "") == "1"
    out, res = _run(
        np.asarray(inputs["x"]),
        np.asarray(inputs["edge_index"]),
        np.asarray(inputs["W0"]),
        np.asarray(inputs["b0"]),
        np.asarray(inputs["W1"]),
        np.asarray(inputs["b1"]),
        np.asarray(inputs["W2"]),
        np.asarray(inputs["b2"]),
        trace=trace,
    )
    if trace and res.exec_time_ns is not None:
        print(f"HW exec time: {res.exec_time_ns} ns")
        if res.instructions_and_trace:
            print(f"trace: {res.instructions_and_trace[1]}")
    return out



# revision 4
# speedup vs baseline: 1.5656x; 1.5656x over previous
"""3-layer GCN (gcn_norm message passing) on 8 Trainium2 NeuronCores.

Architecture (v5):
  - Nodes row-sharded across 8 cores (12500 real + 44 pad rows each); per
    layer each core computes h_mm = relu(h_prev) @ W for its shard, scaled by
    dis[src] (norm factorization: norm = dis[dest]*dis[src]), AllGathers the
    bf16 table, then aggregates messages for the destinations it owns.
  - The per-layer AllGather is split into 4 uneven source chunks of
    [28,28,28,14] blocks; chunk q's collective fires as soon as the groups
    that produce those table rows finish in the previous layer, overlapping
    collective time with compute. Gathers for window q wait only on chunk q.
  - Self-loops are NOT gathered: each block's own table rows are added into
    the aggregation PSUM with an identity matmul (hm rows are already scaled
    by dis[src]; the epilogue's *dis[dest] completes the dis^2 self norm).
  - Messages sorted by (dest-group of 4 blocks, source-chunk, dest-block).
    Per (block, chunk) runs are padded to 32-slot units so run boundaries
    are identical on all cores; one dma_gather per (group, chunk) on 4
    parallel SWDGE queues (int16 indices address the bf16 chunk buffers).
  - Segment-sum on the TensorEngine: each 128-message chunk contributes one
    N=128 matmul per (statically known) destination block it overlaps, into
    a [128, 512] group PSUM tile; one-hots are 0/1 bf16 built 8 chunks at a
    time with a single wide DVE tensor_tensor against a bf16 iota (2x DVE
    rate vs f32).
  - gidx and meta are loaded into SBUF once and sliced per call (no
    per-group reloads).

All data-dependent structure is baked at trace time; the NEFF is compiled
per call and cached in-process.
"""

import os
import sys

sys.path.insert(0, "/opt/trn_rl_repo")

import numpy as np
import ml_dtypes

from concourse import bacc, bass, mybir
from concourse import tile
from concourse import bass_utils

F32 = mybir.dt.float32
BF16 = mybir.dt.bfloat16
I16 = mybir.dt.int16

N_CORES = 8
NQ = 4       # source chunks (gather windows / AllGather chunks)
G = 4        # dest blocks per gather group
WOH = 8      # one-hot chunks per wide DVE op
RUN = 32     # run alignment granularity (slots)
PAD_SEG = 10000.0
CHUNK_BLKS = [28, 28, 28, 14]   # source blocks per chunk (sum = nblk)

SCRATCH = int(os.environ.get("TRN_SCRATCH", "32768"))
GATH_BUFS = int(os.environ.get("TRN_GATH_BUFS", "12"))


def _schedule(caps32, ngrp, nblk):
    """Static layout shared by prep and builder.

    caps32: [nblk][NQ] per-(block, chunk) run capacity in RUN-slot units.
    Returns per-call offsets and the chunk->block matmul schedule.
    """
    call_cols = np.zeros(ngrp * NQ + 1, dtype=np.int64)   # gidx col base
    chunk_base = np.zeros(ngrp * NQ + 1, dtype=np.int64)  # chunk id base
    call_nidx = []
    run_slot = {}   # (b, q) -> slot offset of run inside its call
    mm_of_group = []
    for g in range(ngrp):
        blocks = list(range(g * G, min((g + 1) * G, nblk)))
        mms = []
        for q in range(NQ):
            off = 0
            spans = []
            for b in blocks:
                run_slot[(b, q)] = off
                spans.append((b, off, off + caps32[b][q] * RUN))
                off += caps32[b][q] * RUN
            nidx = ((off + 127) // 128) * 128
            call_nidx.append(nidx)
            call_cols[g * NQ + q + 1] = call_cols[g * NQ + q] + nidx // 16
            chunk_base[g * NQ + q + 1] = chunk_base[g * NQ + q] + nidx // 128
            for c in range(nidx // 128):
                lo, hi = c * 128, (c + 1) * 128
                for b, s0, s1 in spans:
                    if s0 < hi and s1 > lo:
                        mms.append((b - g * G, q, c))
        # j-major order for PSUM accumulation bracketing
        mms.sort(key=lambda m: (m[0], m[1], m[2]))
        mm_of_group.append(mms)
    mm_base = np.zeros(ngrp + 1, dtype=np.int64)
    np.cumsum([len(m) for m in mm_of_group], out=mm_base[1:])
    return {
        "call_cols": call_cols, "chunk_base": chunk_base,
        "call_nidx": call_nidx, "run_slot": run_slot,
        "mm_of_group": mm_of_group, "mm_base": mm_base,
        "n_mm": int(mm_base[-1]),
        "gidx_cols": int(call_cols[-1]),
        "total_chunks": int(chunk_base[-1]),
    }


# ----------------------------------------------------------------------------
# Host-side preparation
# ----------------------------------------------------------------------------

def _prep_inputs(x, edge_index, W0, b0, W1, b1, W2, b2, s_real):
    n = x.shape[0]
    assert n % N_CORES == 0 and s_real == n // N_CORES
    nblk = (s_real + 127) // 128
    s_pad = nblk * 128
    ngrp = (nblk + G - 1) // G
    assert sum(CHUNK_BLKS) == nblk and len(CHUNK_BLKS) == NQ
    chunk_rows = [c * 128 for c in CHUNK_BLKS]
    chunk_base_rows = np.concatenate([[0], np.cumsum(chunk_rows)])
    assert all(N_CORES * r <= 32767 for r in chunk_rows)
    # chunk q must cover whole groups
    chunk_base_blk = np.concatenate([[0], np.cumsum(CHUNK_BLKS)])
    assert all(b % G == 0 for b in chunk_base_blk[:-1])

    d = np.asarray(edge_index[0], dtype=np.int64)
    s = np.asarray(edge_index[1], dtype=np.int64)

    deg = np.bincount(s, minlength=n).astype(np.float64) + 1.0
    dis = (1.0 / np.sqrt(deg)).astype(np.float32)

    core = d // s_real
    dloc = d - core * s_real
    blk = dloc >> 7
    grp = blk // G
    jj = blk - grp * G
    score = s // s_real
    sloc = s - score * s_real
    sblk = sloc >> 7
    blk2chunk = np.zeros(nblk, dtype=np.int64)
    for q in range(NQ):
        blk2chunk[chunk_base_blk[q]:chunk_base_blk[q + 1]] = q
    q = blk2chunk[sblk]
    widx = (
        score * np.asarray(chunk_rows)[q] + (sloc - chunk_base_rows[q])
    ).astype(np.int64)

    # per-(core, block, chunk) counts -> shared run capacities (RUN units)
    key = (core * nblk + blk) * NQ + q
    counts = np.bincount(key, minlength=N_CORES * nblk * NQ).reshape(
        N_CORES, nblk, NQ
    )
    caps32 = np.maximum(
        (counts.max(axis=0) + RUN - 1) // RUN, 1
    )  # [nblk, NQ]

    lay = _schedule(caps32.tolist(), ngrp, nblk)

    # rank within (core, b, q)
    order = np.argsort(key, kind="stable")
    inv = np.empty_like(order)
    inv[order] = np.arange(order.size)
    starts = np.zeros(N_CORES * nblk * NQ + 1, dtype=np.int64)
    np.cumsum(counts.reshape(-1), out=starts[1:])
    rank = inv - starts[key]

    run_slot_arr = np.zeros((nblk, NQ), dtype=np.int64)
    for (b, qq), v in lay["run_slot"].items():
        run_slot_arr[b, qq] = v

    slot = run_slot_arr[blk, q] + rank               # slot within call
    call_id = grp * NQ + q
    gcol = lay["call_cols"][call_id] + (slot >> 4)
    grow = slot & 15
    gchunk = lay["chunk_base"][call_id] + (slot >> 7)
    part = slot & 127

    gidx16 = np.zeros((N_CORES, 16, lay["gidx_cols"]), dtype=np.int16)
    gidx16[core, grow, gcol] = widx.astype(np.int16)
    gidx = np.broadcast_to(
        gidx16[:, None, :, :], (N_CORES, 8, 16, lay["gidx_cols"])
    ).reshape(N_CORES, 128, lay["gidx_cols"]).copy()

    # mm col lookup: (gchunk, j) -> column
    mm_col = np.full((lay["total_chunks"], G), -1, dtype=np.int64)
    for g in range(ngrp):
        m0 = lay["mm_base"][g]
        cb = lay["chunk_base"]
        for k, (j, qq, c) in enumerate(lay["mm_of_group"][g]):
            mm_col[cb[g * NQ + qq] + c, j] = m0 + k

    meta = np.full((N_CORES, 128, lay["n_mm"]), PAD_SEG, dtype=np.float32)
    col = mm_col[gchunk, jj]
    assert (col >= 0).all()
    meta[core, part, col] = (dloc - blk * 128).astype(np.float32)
    meta = meta.astype(ml_dtypes.bfloat16)

    # dense inputs
    x = np.asarray(x, dtype=np.float32)
    x_t = np.zeros((N_CORES, 128, s_pad), dtype=np.float32)
    dison = np.zeros((N_CORES, 128, nblk), dtype=np.float32)
    disd = np.zeros((N_CORES, 128, s_pad), dtype=np.float32)
    for r in range(N_CORES):
        x_t[r, :, :s_real] = x[r * s_real : (r + 1) * s_real].T
        dv = np.zeros(s_pad, dtype=np.float32)
        dv[:s_real] = dis[r * s_real : (r + 1) * s_real]
        dison[r] = dv.reshape(nblk, 128).T
        disd[r] = dv[None, :]

    wdata = np.zeros((128, 3 * 128 + 3 + 128), dtype=np.float32)
    wdata[:, 0:128] = np.asarray(W0, dtype=np.float32)
    wdata[:, 128:256] = np.asarray(W1, dtype=np.float32)
    wdata[:, 256:384] = np.asarray(W2, dtype=np.float32)
    wdata[:, 384] = np.asarray(b0, dtype=np.float32)
    wdata[:, 385] = np.asarray(b1, dtype=np.float32)
    wdata[:, 386] = np.asarray(b2, dtype=np.float32)
    wdata[:, 387:515] = np.eye(128, dtype=np.float32)
    iotar = np.tile(
        np.arange(128, dtype=np.float32), WOH
    )[None, :].repeat(128, axis=0).astype(ml_dtypes.bfloat16)

    in_maps = [
        {
            "x_t": x_t[r], "meta": meta[r], "gidx": gidx[r],
            "wdata": wdata, "iotar": iotar, "dison": dison[r],
            "disd": disd[r],
        }
        for r in range(N_CORES)
    ]
    sched = {
        "nblk": nblk, "s_pad": s_pad, "s_real": s_real, "ngrp": ngrp,
        "caps32": caps32.tolist(),
    }
    return in_maps, sched


# ----------------------------------------------------------------------------
# Device kernel builder
# ----------------------------------------------------------------------------

def build_kernel(sched, n_cores=N_CORES):
    from contextlib import ExitStack

    nblk, s_pad, ngrp = sched["nblk"], sched["s_pad"], sched["ngrp"]
    caps32 = sched["caps32"]
    lay = _schedule(caps32, ngrp, nblk)
    chunk_rows = [c * 128 for c in CHUNK_BLKS]
    chunk_base_rows = np.concatenate([[0], np.cumsum(chunk_rows)])
    chunk_base_blk = np.concatenate([[0], np.cumsum(CHUNK_BLKS)])
    # AG(L+1, q) issued after this group of layer L (q<NQ-1 with slack;
    # last chunk after the loop). Phase A: after slab covering chunk end.
    ag_after_group = {}
    for q in range(NQ - 1):
        g_ready = (chunk_base_blk[q + 1] + G - 1) // G - 1  # chunk written
        ag_after_group[min(g_ready + 2, ngrp - 1)] = q

    nc = bacc.Bacc(
        "TRN2", target_bir_lowering=False, debug=False, num_devices=n_cores,
        num_swdge_queues=NQ, dynamic_dma_scratch_size=SCRATCH,
    )
    x_t = nc.dram_tensor("x_t", [128, s_pad], F32, kind="ExternalInput")
    meta = nc.dram_tensor("meta", [128, lay["n_mm"]], BF16, kind="ExternalInput")
    gidx = nc.dram_tensor("gidx", [128, lay["gidx_cols"]], I16, kind="ExternalInput")
    wdata = nc.dram_tensor("wdata", [128, 3 * 128 + 3 + 128], F32, kind="ExternalInput")
    iotar = nc.dram_tensor("iotar", [128, WOH * 128], BF16, kind="ExternalInput")
    dison = nc.dram_tensor("dison", [128, nblk], F32, kind="ExternalInput")
    disd = nc.dram_tensor("disd", [128, s_pad], F32, kind="ExternalInput")
    h_out = nc.dram_tensor("h_out", [128, 3 * s_pad], F32, kind="ExternalOutput")

    rg = [list(range(n_cores))]
    ID = mybir.ActivationFunctionType

    with tile.TileContext(nc) as tc, ExitStack() as ctx:
        const = ctx.enter_context(tc.tile_pool(name="const", bufs=1))
        dram = ctx.enter_context(tc.tile_pool(name="dram", bufs=1, space="DRAM"))
        xw = ctx.enter_context(tc.tile_pool(name="xw", bufs=4))
        hmm = ctx.enter_context(tc.tile_pool(name="hmm", bufs=4))
        gath = ctx.enter_context(tc.tile_pool(name="gath", bufs=GATH_BUFS))
        ohp = ctx.enter_context(tc.tile_pool(name="ohp", bufs=12))
        hmo = ctx.enter_context(tc.tile_pool(name="hmo", bufs=8))
        outsb = ctx.enter_context(tc.tile_pool(name="outsb", bufs=3))
        ddp = ctx.enter_context(tc.tile_pool(name="ddp", bufs=3))
        rsb = ctx.enter_context(tc.tile_pool(name="rsb", bufs=2))
        agg_ps = ctx.enter_context(tc.tile_pool(name="agg_ps", bufs=2, space="PSUM"))
        mm_ps = ctx.enter_context(tc.tile_pool(name="mm_ps", bufs=2, space="PSUM"))
        mma_ps = ctx.enter_context(tc.tile_pool(name="mma_ps", bufs=2, space="PSUM"))

        ag_ins = [
            dram.tile([s_pad, 128], BF16, name=f"ag_in_l{i}") for i in range(3)
        ]
        ag_outs = [
            [
                dram.tile(
                    [n_cores * chunk_rows[q], 128], BF16,
                    addr_space="Shared", name=f"ag_out_l{i}_c{q}",
                )
                for q in range(NQ)
            ]
            for i in range(3)
        ]

        w_sb = const.tile([128, 3 * 128 + 3 + 128], F32)
        nc.sync.dma_start(out=w_sb[:], in_=wdata[:])
        w_bf = const.tile([128, 3 * 128], BF16)
        nc.vector.tensor_copy(w_bf[:], w_sb[:, 0 : 3 * 128])
        id_bf = const.tile([128, 128], BF16)
        nc.vector.tensor_copy(id_bf[:], w_sb[:, 387:515])
        iota_sb = const.tile([128, WOH * 128], BF16)
        nc.sync.dma_start(out=iota_sb[:], in_=iotar[:])
        dison_sb = const.tile([128, nblk], F32)
        nc.sync.dma_start(out=dison_sb[:], in_=dison[:])
        gidx_sb = const.tile([128, lay["gidx_cols"]], I16)
        nc.sync.dma_start(out=gidx_sb[:], in_=gidx[:])
        meta_sb = const.tile([128, lay["n_mm"]], BF16)
        nc.sync.dma_start(out=meta_sb[:], in_=meta[:])

        def bias(L):
            return w_sb[:, 384 + L : 385 + L]

        def issue_ag(L, q):
            r0, r1 = int(chunk_base_rows[q]), int(chunk_base_rows[q + 1])
            nc.gpsimd.collective_compute(
                "AllGather",
                mybir.AluOpType.bypass,
                replica_groups=rg,
                ins=[ag_ins[L][r0:r1, :].opt()],
                outs=[ag_outs[L][q][:].opt()],
            )

        # ---- Phase A: table0 = (x @ W0) * dis -> ag_in0, chunked AGs ----
        nslab = (nblk + 3) // 4
        ag_after_slab = {
            (int(chunk_base_blk[q + 1]) + 3) // 4 - 1: q for q in range(NQ)
        }
        for sl in range(nslab):
            b0 = sl * 4
            bn = min(4, nblk - b0)
            w = bn * 128
            xt = xw.tile([128, 512], F32, name="xt", tag="xt")
            nc.sync.dma_start(out=xt[:, :w], in_=x_t[:, b0 * 128 : b0 * 128 + w])
            ps = mma_ps.tile([128, 512], F32, name="psA", tag="psA")
            for j in range(bn):
                nc.tensor.matmul(
                    ps[:, j * 128 : (j + 1) * 128],
                    lhsT=xt[:, j * 128 : (j + 1) * 128],
                    rhs=w_sb[:, 0:128], start=True, stop=True,
                    skip_group_check=True,
                )
            hm = hmm.tile([128, 512], BF16, name="hmA", tag="hm")
            for j in range(bn):
                b = b0 + j
                nc.scalar.activation(
                    hm[:, j * 128 : (j + 1) * 128],
                    ps[:, j * 128 : (j + 1) * 128],
                    ID.Copy, scale=dison_sb[:, b : b + 1],
                )
                nc.scalar.dma_start(
                    out=ag_ins[0][b * 128 : (b + 1) * 128, :],
                    in_=hm[:, j * 128 : (j + 1) * 128],
                )
            if sl in ag_after_slab:
                issue_ag(0, ag_after_slab[sl])

        # ---- 3 layers ----
        for L in range(3):
            for g in range(ngrp):
                blocks = list(range(g * G, min((g + 1) * G, nblk)))
                nj = len(blocks)
                gts = []
                for q in range(NQ):
                    nidx = lay["call_nidx"][g * NQ + q]
                    c0 = lay["call_cols"][g * NQ + q]
                    gt = gath.tile([128, nidx], BF16, name="gt", tag="gt")
                    nc.gpsimd.dma_gather(
                        gt[:].rearrange("p (c f) -> p c f", f=128),
                        ag_outs[L][q][:],
                        gidx_sb[:, c0 : c0 + nidx // 16],
                        num_idxs=nidx,
                        num_idxs_reg=nidx,
                        elem_size=128,
                        elem_step=128,
                        single_packet=False,
                        queue_num=q,
                    )
                    gts.append(gt)
                if L < 2 and g in ag_after_group:
                    issue_ag(L + 1, ag_after_group[g])

                mms = lay["mm_of_group"][g]
                m0 = int(lay["mm_base"][g])
                n_mm_g = len(mms)

                ohs = {}
                for w0 in range(0, n_mm_g, WOH):
                    wn = min(WOH, n_mm_g - w0)
                    oh = ohp.tile([128, wn * 128], BF16, name="oh", tag="oh")
                    nc.vector.tensor_tensor(
                        oh[:].rearrange("p (c f) -> p c f", f=128),
                        iota_sb[:, : wn * 128].rearrange("p (c f) -> p c f", f=128),
                        meta_sb[:, m0 + w0 : m0 + w0 + wn].to_broadcast(
                            [128, wn, 128]
                        ),
                        mybir.AluOpType.is_equal,
                    )
                    ohs[w0] = oh

                # own-table rows for the self-loop identity matmuls
                hms = []
                for j in range(nj):
                    b = blocks[j]
                    hmj = hmo.tile([128, 128], BF16, name="hmo", tag="hmo")
                    nc.sync.dma_start(
                        out=hmj[:], in_=ag_ins[L][b * 128 : (b + 1) * 128, :]
                    )
                    hms.append(hmj)

                ps = agg_ps.tile([128, G * 128], F32, name="aggps", tag="aggps")
                # strictly sequential PSUM brackets: per block j, open with
                # the self-loop identity matmul, accumulate j's edge mms,
                # close on the last one.
                ptr = 0
                for j in range(nj):
                    nc.tensor.matmul(
                        ps[:, j * 128 : (j + 1) * 128],
                        lhsT=hms[j][:],
                        rhs=id_bf[:],
                        start=True,
                        stop=False,
                        skip_group_check=True,
                    )
                    while ptr < len(mms) and mms[ptr][0] == j:
                        k = ptr
                        _, qq, c = mms[k]
                        w0 = (k // WOH) * WOH
                        off = k - w0
                        last = (k + 1 == len(mms)) or (mms[k + 1][0] != j)
                        nc.tensor.matmul(
                            ps[:, j * 128 : (j + 1) * 128],
                            lhsT=gts[qq][:, c * 128 : (c + 1) * 128],
                            rhs=ohs[w0][:, off * 128 : (off + 1) * 128],
                            start=False,
                            stop=last,
                            skip_group_check=True,
                        )
                        ptr += 1
                # ---- group epilogue ----
                w = nj * 128
                gb = g * G * 128
                dd = ddp.tile([128, G * 128], F32, name="dd", tag="dd")
                nc.scalar.dma_start(out=dd[:, :w], in_=disd[:, gb : gb + w])
                ob = outsb.tile([128, G * 128], F32, name="ob", tag="ob")
                nc.vector.tensor_tensor(
                    ob[:, :w], ps[:, :w], dd[:, :w], mybir.AluOpType.mult
                )
                nc.scalar.activation(ob[:, :w], ob[:, :w], ID.Identity, bias=bias(L))
                nc.sync.dma_start(
                    out=h_out[:, L * s_pad + gb : L * s_pad + gb + w],
                    in_=ob[:, :w],
                )
                if L < 2:
                    r = rsb.tile([128, G * 128], BF16, name="r", tag="r")
                    nc.scalar.activation(r[:, :w], ob[:, :w], ID.Relu)
                    ps2 = mm_ps.tile([128, G * 128], F32, name="ps2", tag="ps2")
                    for j in range(nj):
                        nc.tensor.matmul(
                            ps2[:, j * 128 : (j + 1) * 128],
                            lhsT=r[:, j * 128 : (j + 1) * 128],
                            rhs=w_bf[:, (L + 1) * 128 : (L + 2) * 128],
                            start=True,
                            stop=True,
                            skip_group_check=True,
                        )
                    hm = hmm.tile([128, G * 128], BF16, name="hm", tag="hm")
                    for j in range(nj):
                        b = blocks[j]
                        nc.scalar.activation(
                            hm[:, j * 128 : (j + 1) * 128],
                            ps2[:, j * 128 : (j + 1) * 128], ID.Copy,
                            scale=dison_sb[:, b : b + 1],
                        )
                        nc.scalar.dma_start(
                            out=ag_ins[L + 1][b * 128 : (b + 1) * 128, :],
                            in_=hm[:, j * 128 : (j + 1) * 128],
                        )
            if L < 2:
                issue_ag(L + 1, NQ - 1)

    nc.compile()
    return nc


_BUILD_CACHE = {}


def _get_kernel(sched):
    key = (
        sched["nblk"], sched["s_pad"],
        tuple(tuple(c) for c in sched["caps32"]),
    )
    if key not in _BUILD_CACHE:
        _BUILD_CACHE[key] = build_kernel(sched)
    return _BUILD_CACHE[key]


# ----------------------------------------------------------------------------
# Entry point
# ----------------------------------------------------------------------------

def _run(x, edge_index, W0, b0, W1, b1, W2, b2, trace=False):
    n = int(np.asarray(x).shape[0])
    s_real = n // N_CORES
    in_maps, sched = _prep_inputs(
        x, edge_index, W0, b0, W1, b1, W2, b2, s_real
    )
    s_pad = sched["s_pad"]
    nc = _get_kernel(sched)
    res = bass_utils.run_bass_kernel_spmd(
        nc, in_maps, core_ids=list(range(N_CORES)), trace=trace
    )
    outs = []
    for L in range(3):
        h = np.concatenate(
            [
                res.results[r]["h_out"][:, L * s_pad : L * s_pad + s_real]
                for r in range(N_CORES)
            ],
            axis=1,
        ).T
        outs.append(h)
    full = np.stack(outs, axis=1).astype(np.float32)
    return full, res


def kernel(**inputs):
    trace = os.environ.get("TRN_KERNEL_TRACE", "") == "1"
    out, res = _run(
        np.asarray(inputs["x"]),
        np.asarray(inputs["edge_index"]),
        np.asarray(inputs["W0"]),
        np.asarray(inputs["b0"]),
        np.asarray(inputs["W1"]),
        np.asarray(inputs["b1"]),
        np.asarray(inputs["W2"]),
        np.asarray(inputs["b2"]),
        trace=trace,
    )
    if trace and res.exec_time_ns is not None:
        print(f"HW exec time: {res.exec_time_ns} ns")
        if res.instructions_and_trace:
            print(f"trace: {res.instructions_and_trace[1]}")
    return out
